# revision 30
# baseline (speedup 1.0000x reference)
"""Trainium2 Bass kernel for the BNN/GLIF recurrent network (nn_BNNFC).

Strategy: 8-way tensor parallelism over the hidden dimension H=2048
(256 rows per core). The recurrence over T=512 steps is sequential; each
step does, per core:
  - syn psum = W_ix_shard @ x_t  +  W_hh_shard @ firing_full   (PE, bf16)
  - GLIF elementwise updates (DVE/ACT, f32 state)
  - firing shard -> AllGather across 8 cores -> full firing (h-major)
  - out_t[:, o_shard] = W_out_shard @ firing_full (PE) -> DRAM
Host side shards/transposes inputs and assembles the output.

Layouts (per core, h_local = 256 = 2 m-tiles of 128):
  global h = core*256 + ho*128 + p   (ho in {0,1}, p in [0,128))
  state tiles: [128(p), 2(ho), 64(b)] f32
  firing_full SBUF: [128(p), 8(core), 2(ho), 64(b)] bf16  (16 k-tiles)
"""
import sys, os, time
sys.path.insert(0, "/opt/trn_rl_repo")
import numpy as np

import concourse.bass as bass
import concourse.mybir as mybir
import concourse.tile as tile
from concourse import bacc
from concourse import bass_utils
from concourse.instruction_name_ordered_set import InstructionNameOrderedSet


def _names(*insts):
    s = InstructionNameOrderedSet()
    for i in insts:
        s.add(i.ins.name)
    return s


F32 = mybir.dt.float32
BF16 = mybir.dt.bfloat16

IN, HID, OUT, A = 512, 2048, 512, 2
B, T = 64, 512
DT = 0.05
NC_N = 8            # cores
HL = HID // NC_N    # 256 h rows per core
HO = HL // 128      # 2 m-tiles
OL = OUT // NC_N    # 64 out features per core
NK = HID // 128     # 16 k-tiles over full H
NKI = IN // 128     # 4 k-tiles over input dim


BL = B // NC_N      # 8 batch per core in rep mode
NMO = HID // 128    # 16 m-tiles over full H
NOO = OUT // 128    # 4 out m-tiles


def build_rep(t_steps=T):
    """Replicated weights + batch data-parallel: no collectives at all.
    Every core holds full f32 W_hh/W_ix (LDW-bound, col-tiled) and full
    bf16 W_out (no feedback -> bf16 safe), and runs the whole recurrence
    for its 8-sample batch shard."""
    nc = bacc.Bacc("TRN2", target_bir_lowering=False, debug=False,
                   num_devices=NC_N)
    xT_d = nc.dram_tensor("xT", [t_steps, 128, NKI, BL], F32,
                          kind="ExternalInput")
    wix_d = nc.dram_tensor("wix", [128, NKI, NMO, 128], F32,
                           kind="ExternalInput")
    whh_d = nc.dram_tensor("whh", [128, NK, NMO, 128], F32,
                           kind="ExternalInput")
    wout_d = nc.dram_tensor("wout", [128, NK, NOO, 128], BF16,
                            kind="ExternalInput")
    # full-H broadcast param tiles [128, NMO, BL]
    pnames = ["r1t", "r2t", "amp1t", "amp2t", "dec1t", "dec2t", "km1t",
              "negtht", "bivt", "boutt"]
    pds = {n: nc.dram_tensor(n, [128, NOO if n == "boutt" else NMO, BL],
                             F32, kind="ExternalInput") for n in pnames}
    out_d = nc.dram_tensor("out", [t_steps, 128, NOO, BL], F32,
                           kind="ExternalOutput")

    with tile.TileContext(nc) as tc:
        with (
            tc.tile_pool(name="static", bufs=1) as sp,
            tc.tile_pool(name="state", bufs=1) as st,
            tc.tile_pool(name="fire", bufs=2) as fp,
            tc.tile_pool(name="xin", bufs=3) as xp,
            tc.tile_pool(name="tmp", bufs=2) as tp,
            tc.tile_pool(name="outs", bufs=2) as op_,
            tc.tile_pool(name="psyn", bufs=2, space="PSUM") as pps,
            tc.tile_pool(name="pout", bufs=2, space="PSUM") as ppo,
        ):
            wix = sp.tile([128, NKI, NMO, 128], F32)
            whh = sp.tile([128, NK, NMO, 128], F32)
            wout = sp.tile([128, NK, NOO, 128], BF16)
            nc.sync.dma_start(wix[:], wix_d[:])
            nc.sync.dma_start(whh[:], whh_d[:])
            nc.sync.dma_start(wout[:], wout_d[:])
            pt = {}
            for n in pnames:
                pt[n] = sp.tile([128, NOO if n == "boutt" else NMO, BL],
                                F32, name=n, tag=n)
                nc.sync.dma_start(pt[n][:], pds[n][:])

            v = st.tile([128, NMO, BL], F32)
            a1 = st.tile([128, NMO, BL], F32)
            a2 = st.tile([128, NMO, BL], F32)
            nc.vector.memset(v[:], 0.0)
            nc.vector.memset(a1[:], 0.0)
            nc.vector.memset(a2[:], 0.0)
            fire = fp.tile([128, NMO, BL], F32, tag="fire")
            nc.gpsimd.memset(fire[:], 0.0)

            for t in range(t_steps):
                xt = xp.tile([128, NKI, BL], F32, tag="xt")
                nc.sync.dma_start(xt[:], xT_d[t])

                psyn = pps.tile([128, NMO, BL], F32, tag="psyn")
                for mo in range(NMO):
                    for hf in range(2):
                        sl = slice(hf * 64, (hf + 1) * 64)
                        for ki in range(NKI):
                            nc.tensor.matmul(
                                psyn[sl, mo, :], wix[:, ki, mo, sl],
                                xt[:, ki, :], start=(ki == 0), stop=False,
                                tile_position=(0, hf * 64),
                                skip_group_check=True)
                        for kidx in range(NK):
                            nc.tensor.matmul(
                                psyn[sl, mo, :], whh[:, kidx, mo, sl],
                                fire[:, kidx, :], start=False,
                                stop=(kidx == NK - 1),
                                tile_position=(0, hf * 64),
                                skip_group_check=True)

                # GLIF, full-tile [128, NMO*BL] f32 with broadcast params
                u1 = tp.tile([128, NMO, BL], F32, tag="u1")
                u2 = tp.tile([128, NMO, BL], F32, tag="u2")
                kv = tp.tile([128, NMO, BL], F32, tag="kv")
                fv = tp.tile([128, NMO, BL], F32, tag="fv")
                pre = tp.tile([128, NMO, BL], F32, tag="pre")
                sg = tp.tile([128, NMO, BL], F32, tag="sg")
                nc.vector.tensor_mul(u1[:], a1[:], pt["r1t"][:])
                nc.vector.tensor_add(u1[:], u1[:], pt["amp1t"][:])
                nc.vector.tensor_mul(u2[:], a2[:], pt["r2t"][:])
                nc.vector.tensor_add(u2[:], u2[:], pt["amp2t"][:])
                nc.vector.tensor_mul(u1[:], u1[:], fire[:])
                nc.vector.tensor_mul(u2[:], u2[:], fire[:])
                nc.vector.tensor_mul(a1[:], a1[:], pt["dec1t"][:])
                nc.vector.tensor_add(a1[:], a1[:], u1[:])
                nc.vector.tensor_mul(a2[:], a2[:], pt["dec2t"][:])
                nc.vector.tensor_add(a2[:], a2[:], u2[:])
                nc.vector.tensor_mul(kv[:], v[:], pt["km1t"][:])
                nc.vector.tensor_mul(fv[:], v[:], fire[:])
                nc.vector.tensor_add(pre[:], a1[:], a2[:])
                nc.vector.tensor_add(pre[:], pre[:], pt["bivt"][:])
                nc.vector.tensor_add(pre[:], pre[:], kv[:])
                nc.vector.tensor_tensor(
                    pre[:], pre[:], fv[:], mybir.AluOpType.subtract)
                nc.vector.tensor_add(v[:], psyn[:], pre[:])
                fire_new = fp.tile([128, NMO, BL], F32, tag="fire")
                nc.vector.tensor_add(sg[:], v[:], pt["negtht"][:])
                nc.scalar.activation(
                    fire_new[:], sg[:],
                    mybir.ActivationFunctionType.Sigmoid,
                    bias=0.0, scale=1.0)
                fb = fp.tile([128, NMO, BL], BF16, tag="fireb")
                nc.vector.tensor_copy(fb[:], fire_new[:])

                pout = ppo.tile([128, NOO, BL], F32, tag="pout")
                for mo in range(NOO):
                    for kidx in range(NK):
                        nc.tensor.matmul(
                            pout[:, mo, :], wout[:, kidx, mo, :],
                            fb[:, kidx, :],
                            start=(kidx == 0), stop=(kidx == NK - 1),
                            skip_group_check=True)
                osb = op_.tile([128, NOO, BL], F32, tag="osb")
                nc.vector.tensor_add(osb[:], pout[:], pt["boutt"][:])
                nc.sync.dma_start(out_d[t], osb[:])
                fire = fire_new

    nc.compile()
    return nc


def prep_inputs_rep(inp, W_iv, b_iv, W_out, b_out, thresh, k_m, asc_amp,
                    asc_r, asc_k, t_steps=T):
    inp = np.asarray(inp, np.float32)
    W_iv = np.asarray(W_iv, np.float32)
    b_iv = np.asarray(b_iv, np.float32).reshape(HID)
    W_out = np.asarray(W_out, np.float32)
    b_out = np.asarray(b_out, np.float32).reshape(OUT)
    thresh = np.asarray(thresh, np.float32).reshape(HID)
    k_m = np.asarray(k_m, np.float32).reshape(HID)
    asc_amp = np.asarray(asc_amp, np.float32).reshape(A, HID)
    asc_r = np.asarray(asc_r, np.float32).reshape(A, HID)
    asc_k = np.asarray(asc_k, np.float32).reshape(A, HID)
    W_ix = W_iv[:, :IN]
    W_hh = W_iv[:, IN:]
    dec = np.exp(np.float32(-DT) * asc_k).astype(np.float32)
    km1 = (1.0 - np.float32(DT) * k_m).astype(np.float32)
    import ml_dtypes
    # lhsT layouts: [p_k, kt, mo, p_m]  (h = mo*128 + p_m, k = kt*128 + p_k)
    wix = np.ascontiguousarray(
        W_ix.reshape(NMO, 128, NKI, 128).transpose(3, 2, 0, 1), np.float32)
    whh = np.ascontiguousarray(
        W_hh.reshape(NMO, 128, NK, 128).transpose(3, 2, 0, 1), np.float32)
    wout = np.ascontiguousarray(
        W_out.reshape(NOO, 128, NK, 128).transpose(3, 2, 0, 1)).astype(
            ml_dtypes.bfloat16)
    bch = lambda x, bl: np.repeat(           # [H]->[128, NMO, BL]
        x.reshape(-1, 128).T[:, :, None], bl, axis=2).astype(np.float32)
    shared = {
        "wix": wix, "whh": whh, "wout": wout,
        "r1t": bch(asc_r[0], BL), "r2t": bch(asc_r[1], BL),
        "amp1t": bch(asc_amp[0], BL), "amp2t": bch(asc_amp[1], BL),
        "dec1t": bch(dec[0], BL), "dec2t": bch(dec[1], BL),
        "km1t": bch(km1, BL), "negtht": bch(-thresh, BL),
        "bivt": bch(b_iv, BL), "boutt": bch(b_out, BL),
    }
    in_maps = []
    for c in range(NC_N):
        bs = slice(c * BL, (c + 1) * BL)
        # xT[t, p, ki, b] = inp[b, t, ki*128+p]
        xT = np.ascontiguousarray(
            inp[bs, :t_steps, :].transpose(1, 2, 0).reshape(
                t_steps, NKI, 128, BL).transpose(0, 2, 1, 3), np.float32)
        in_maps.append({**shared, "xT": xT})
    return in_maps


def assemble_output_rep(results, t_steps=T):
    full = np.empty((B, t_steps, OUT), np.float32)
    for c, r in enumerate(results):
        o = r["out"]                       # [t, 128, NOO, BL]
        # o[t, p, mo, b] -> full[c*BL+b, t, mo*128+p]
        full[c * BL:(c + 1) * BL] = o.transpose(3, 0, 2, 1).reshape(
            BL, t_steps, OUT)
    return full


def build(t_steps=T, mode='full'):
    nc = bacc.Bacc("TRN2", target_bir_lowering=False, debug=False,
                   num_devices=NC_N, enable_partition_id=(mode == 'rdma'))

    # ---- external inputs (per-core values supplied via in_maps) ----
    xT_d = nc.dram_tensor("xT", [IN, t_steps, B], F32, kind="ExternalInput")
    wix_d = nc.dram_tensor("wix", [128, NKI, HO, 128], F32, kind="ExternalInput")
    whh_d = nc.dram_tensor("whh", [128, NK, HO, 128], F32, kind="ExternalInput")
    wout_d = nc.dram_tensor("wout", [128, NK, OL], F32, kind="ExternalInput")
    # per-partition param columns [128, HO] and broadcast tiles [128, HO, B]
    rcol_d = nc.dram_tensor("rcol", [128, A, HO], F32, kind="ExternalInput")
    deccol_d = nc.dram_tensor("deccol", [128, A, HO], F32, kind="ExternalInput")
    ampt_d = nc.dram_tensor("ampt", [128, A, HO, B], F32, kind="ExternalInput")
    km1t_d = nc.dram_tensor("km1t", [128, HO, B], F32, kind="ExternalInput")
    negth_d = nc.dram_tensor("negth", [128, HO], F32, kind="ExternalInput")
    bivt_d = nc.dram_tensor("bivt", [128, HO, B], F32, kind="ExternalInput")
    boutcol_d = nc.dram_tensor("boutcol", [OL, 1], F32, kind="ExternalInput")

    out_d = nc.dram_tensor("out", [t_steps, OL, B], F32, kind="ExternalOutput")

    with tile.TileContext(nc) as tc:
        with (
            tc.tile_pool(name="static", bufs=1) as sp,
            tc.tile_pool(name="state", bufs=1) as st,
            tc.tile_pool(name="fire", bufs=2) as fp,
            tc.tile_pool(name="send", bufs=2) as sd,
            tc.tile_pool(name="xin", bufs=3) as xp,
            tc.tile_pool(name="tmp", bufs=2) as tp,
            tc.tile_pool(name="outs", bufs=2) as op_,
            tc.tile_pool(name="psyn", bufs=2, space="PSUM") as pps,
            tc.tile_pool(name="pout", bufs=2, space="PSUM") as ppo,
            tc.tile_pool(name="dram", bufs=2, space="DRAM") as dp,
        ):
            # ---- load static weights/params into SBUF ----
            wix = sp.tile([128, NKI, HO, 128], F32)
            whh = sp.tile([128, NK, HO, 128], F32)
            wout = sp.tile([128, NK, OL], F32)
            rcol = sp.tile([128, A, HO], F32)
            deccol = sp.tile([128, A, HO], F32)
            ampt = sp.tile([128, A, HO, B], F32)
            km1t = sp.tile([128, HO, B], F32)
            negth = sp.tile([128, HO], F32)
            bivt = sp.tile([128, HO, B], F32)
            boutcol = sp.tile([OL, 1], F32)
            nc.sync.dma_start(wix[:], wix_d[:])
            nc.sync.dma_start(whh[:], whh_d[:])
            nc.sync.dma_start(wout[:], wout_d[:])
            nc.sync.dma_start(rcol[:], rcol_d[:])
            nc.sync.dma_start(deccol[:], deccol_d[:])
            nc.sync.dma_start(ampt[:], ampt_d[:])
            nc.sync.dma_start(km1t[:], km1t_d[:])
            nc.sync.dma_start(negth[:], negth_d[:])
            nc.sync.dma_start(bivt[:], bivt_d[:])
            nc.sync.dma_start(boutcol[:], boutcol_d[:])

            # ---- persistent state (f32), zero-init ----
            v = st.tile([128, HO, B], F32)
            a1 = st.tile([128, HO, B], F32)
            a2 = st.tile([128, HO, B], F32)
            fire32 = st.tile([128, HO, B], F32)   # zeros; step-0 "prev firing"
            nc.vector.memset(v[:], 0.0)
            nc.vector.memset(a1[:], 0.0)
            nc.vector.memset(a2[:], 0.0)
            nc.vector.memset(fire32[:], 0.0)
            send_prev = fire32

            fire_prev = fp.tile([128, NC_N, HO, B], F32, tag="fire")
            nc.gpsimd.memset(fire_prev[:], 0.0)

            if mode == 'rdma':
                rsem = nc.alloc_semaphore("rsem")
                lsem = nc.alloc_semaphore("lsem")
                wkeep = nc.alloc_semaphore("wkeep")
                pid = nc.gpsimd.partition_id()
                nc.gpsimd.sem_clear(rsem)
                nc.gpsimd.sem_clear(lsem)
                # cross-core startup barrier: nobody broadcasts until every
                # core has cleared its semaphores (stale values persist
                # across executions of the same loaded NEFF).
                bar_i = dp.tile([1, 4], F32, tag="bar_i")
                bar_o = dp.tile([NC_N, 1, 4], F32, tag="bar_o")
                bar_inst = nc.gpsimd.collective_compute(
                    "AllGather", mybir.AluOpType.bypass,
                    ins=[bar_i[:].opt()], outs=[bar_o[:].opt()],
                    replica_groups=[list(range(NC_N))])
                pout_last = None
                prev_sig = None
                wait_fixups = []  # (inst_name, sem, value): bump post-compile

            for t in range(t_steps):
                # -- prefetch x_t (bf16 k-tiles) --
                xt = xp.tile([128, NKI, B], F32, tag="xt")
                nc.sync.dma_start(
                    xt[:],
                    xT_d.ap().rearrange("(ki p) tt b -> p ki tt b",
                                        p=128)[:, :, t, :])

                # -- syn matmuls into psum [128, HO, B] --
                # col-tiled matmul pairs: two 64-column stationary loads
                # stream via separate XBUSes, halving the LDWEIGHTS-bound
                # W traversal (LDW cost is per-column).
                psyn = pps.tile([128, HO, B], F32, tag="psyn")
                for ho in range(HO):
                    for ki in range(NKI):
                        for hf in range(2):
                            mm = nc.tensor.matmul(
                                psyn[hf * 64:(hf + 1) * 64, ho, :],
                                wix[:, ki, ho, hf * 64:(hf + 1) * 64],
                                xt[:, ki, :],
                                start=(ki == 0), stop=False,
                                tile_position=(0, hf * 64),
                                skip_group_check=True)
                            if (ki == 0 and hf == 0 and mode == 'rdma'
                                    and pout_last is not None):
                                mm.ins.add_nosync_dependencies_from(
                                    _names(pout_last))
                    for c in range(NC_N):
                        for ho2 in range(HO):
                            kidx = c * HO + ho2
                            last = (c == NC_N - 1 and ho2 == HO - 1)
                            for hf in range(2):
                                mm = nc.tensor.matmul(
                                    psyn[hf * 64:(hf + 1) * 64, ho, :],
                                    whh[:, kidx, ho, hf * 64:(hf + 1) * 64],
                                    fire_prev[:, c, ho2, :],
                                    start=False, stop=last,
                                    tile_position=(0, hf * 64),
                                    skip_group_check=True)
                            syn_last = mm

                # -- GLIF elementwise --
                # IEEE-faithful to the reference (inf/NaN propagation must
                # match: e.g. 0*inf=NaN in km1*v - f*v, so no factoring).
                # off-critical-path pieces (depend only on prev state):
                if mode == 'noglif':
                    send = sd.tile([128, HO, B], F32, tag="send")
                    for ho in range(HO):
                        nc.scalar.activation(
                            send[:, ho, :], psyn[:, ho, :],
                            mybir.ActivationFunctionType.Sigmoid,
                            bias=negth[:, ho:ho + 1], scale=1.0)
                    fire_new = fire_prev
                    pout = ppo.tile([OL, B], F32, tag="pout")
                    for kidx in range(NK):
                        nc.tensor.matmul(
                            pout[:], wout[:, kidx, :],
                            fire_new[:, kidx // HO, kidx % HO, :],
                            start=(kidx == 0), stop=(kidx == NK - 1))
                    osb = op_.tile([OL, B], F32, tag="osb")
                    nc.scalar.activation(
                        osb[:], pout[:], mybir.ActivationFunctionType.Identity,
                        bias=boutcol[:], scale=1.0)
                    nc.sync.dma_start(out_d[t], osb[:])
                    continue
                u1 = tp.tile([128, HO, B], F32, tag="u1")
                u2 = tp.tile([128, HO, B], F32, tag="u2")
                kv = tp.tile([128, HO, B], F32, tag="kv")
                fv = tp.tile([128, HO, B], F32, tag="fv")
                pre = tp.tile([128, HO, B], F32, tag="pre")
                for ho in range(HO):
                    # u_k = r_k * a_k + amp_k
                    nc.vector.scalar_tensor_tensor(
                        u1[:, ho, :], a1[:, ho, :], rcol[:, 0, ho:ho + 1],
                        ampt[:, 0, ho, :], mybir.AluOpType.mult,
                        mybir.AluOpType.add)
                    nc.vector.scalar_tensor_tensor(
                        u2[:, ho, :], a2[:, ho, :], rcol[:, 1, ho:ho + 1],
                        ampt[:, 1, ho, :], mybir.AluOpType.mult,
                        mybir.AluOpType.add)
                # m_k = f * u_k ; a_k' = a_k*dec_k + m_k
                nc.vector.tensor_mul(u1[:], u1[:], send_prev[:])
                nc.vector.tensor_mul(u2[:], u2[:], send_prev[:])
                for ho in range(HO):
                    nc.vector.scalar_tensor_tensor(
                        a1[:, ho, :], a1[:, ho, :], deccol[:, 0, ho:ho + 1],
                        u1[:, ho, :], mybir.AluOpType.mult, mybir.AluOpType.add)
                    nc.vector.scalar_tensor_tensor(
                        a2[:, ho, :], a2[:, ho, :], deccol[:, 1, ho:ho + 1],
                        u2[:, ho, :], mybir.AluOpType.mult, mybir.AluOpType.add)
                # pre = a1' + a2' + b_iv + km1*v - f*v  (unfactored)
                nc.vector.tensor_mul(kv[:], v[:], km1t[:])
                nc.vector.tensor_mul(fv[:], v[:], send_prev[:])
                nc.vector.tensor_add(pre[:], a1[:], a2[:])
                nc.vector.tensor_add(pre[:], pre[:], bivt[:])
                nc.vector.tensor_add(pre[:], pre[:], kv[:])
                nc.vector.tensor_tensor(
                    pre[:], pre[:], fv[:], mybir.AluOpType.subtract)
                # critical: v = psum + pre ; f = sigmoid(v - thresh)
                nc.vector.tensor_add(v[:], psyn[:], pre[:])
                send = sd.tile([128, HO, B], F32, tag="send")
                wact = None
                if mode == 'rdma' and t >= 2:
                    wact = nc.scalar.wait_ge(lsem, 0).then_inc(wkeep, 1)
                    wait_fixups.append((wact.ins.name, lsem, 16 * (t - 1)))
                    if prev_sig is not None:
                        wact.ins.add_nosync_dependencies_from(_names(prev_sig))
                for ho in range(HO):
                    si = nc.scalar.activation(
                        send[:, ho, :], v[:, ho, :],
                        mybir.ActivationFunctionType.Sigmoid,
                        bias=negth[:, ho:ho + 1], scale=1.0)
                    if wact is not None:
                        si.ins.add_nosync_dependencies_from(_names(wact))
                    prev_sig = si
                send_prev = send

                # -- allgather firing across cores --
                if mode == 'rdma':
                    fire_new = fp.tile([128, NC_N, HO, B], F32, tag="fire")
                    prep = nc.gpsimd.remote_dma_broadcast(
                        out_ap=fire_new[:, bass.ds(pid, 1), :, :],
                        in_ap=send[:],
                        remote_sem=rsem, local_sem=lsem,
                        rdests=[(0, k) for k in range(NC_N)])
                    if t == 0:
                        prep.ins.add_nosync_dependencies_from(
                            _names(bar_inst))
                    trig = nc.gpsimd.trigger_dma(count=None)
                    wpe = nc.tensor.wait_ge(rsem, 0).then_inc(wkeep, 1)
                    wait_fixups.append((wpe.ins.name, rsem, 16 * (t + 1)))
                    wpe.ins.add_nosync_dependencies_from(_names(syn_last))
                elif mode != 'noag':
                    # ONE AllGather per step: each collective has a ~13.5 us
                    # fixed cost here (measured: splitting into two 32KB AGs
                    # costs +13.5 us/step), so minimize collective count.
                    in_b = dp.tile([128, HO, B], F32, tag="agin")
                    out_b = dp.tile([NC_N, 128, HO, B], F32, tag="agout")
                    nc.sync.dma_start(in_b[:], send[:])
                    nc.gpsimd.collective_compute(
                        "AllGather", mybir.AluOpType.bypass,
                        ins=[in_b[:].opt()], outs=[out_b[:].opt()],
                        replica_groups=[list(range(NC_N))])
                    fire_new = fp.tile([128, NC_N, HO, B], F32, tag="fire")
                    rb = out_b[:].rearrange("c p ho b -> p c ho b")
                    for q in range(NC_N):
                        nc.sync.dma_start(
                            fire_new[:, q:q + 1, :, :],
                            rb[:, q:q + 1, :, :])
                else:
                    fire_new = fire_prev

                # -- output projection for this step (uses gathered fire_new) --
                pout = ppo.tile([OL, B], F32, tag="pout")
                for kidx in range(NK):
                    mm = nc.tensor.matmul(
                        pout[:], wout[:, kidx, :],
                        fire_new[:, kidx // HO, kidx % HO, :],
                        start=(kidx == 0), stop=(kidx == NK - 1))
                    if mode == 'rdma':
                        if kidx == 0:
                            mm.ins.add_nosync_dependencies_from(_names(wpe))
                        if kidx == NK - 1:
                            pout_last = mm
                osb = op_.tile([OL, B], F32, tag="osb")
                nc.scalar.activation(
                    osb[:], pout[:], mybir.ActivationFunctionType.Identity,
                    bias=boutcol[:], scale=1.0)
                nc.sync.dma_start(out_d[t], osb[:])

                fire_prev = fire_new

            if mode == 'rdma':
                wq1 = nc.gpsimd.wait_ge(rsem, 0).then_inc(wkeep, 1)
                wq2 = nc.gpsimd.wait_ge(lsem, 0).then_inc(wkeep, 1)
                wq1.ins.add_nosync_dependencies_from(_names(trig))
                wq2.ins.add_nosync_dependencies_from(_names(wq1))
                wait_fixups.append((wq1.ins.name, rsem, 16 * t_steps))
                wait_fixups.append((wq2.ins.name, lsem, 16 * t_steps))

    nc.compile()
    if mode == 'rdma':
        fix = {n: (sem, val) for n, sem, val in wait_fixups}
        done = 0
        for f in nc.m.functions:
            for bb in f.blocks:
                for i in bb.instructions:
                    if i.name in fix:
                        sem, val = fix[i.name]
                        w = mybir.SyncWait(
                            sync_type='semaphore', id=sem.num,
                            ant_name=sem.name, wait_mode='sem-ge-imm',
                            wait_value=val)
                        si = i.sync_info
                        keep = [x for x in (si.on_wait if si else [])
                                if not (x.id == sem.num)]
                        i.sync_info = mybir.SyncInfo(
                            on_wait=keep + [w],
                            on_update=list(si.on_update) if si else [])
                        done += 1
        assert done == len(fix), f"wait fixups applied {done}/{len(fix)}"
    return nc


# ---------------- host-side data prep ----------------
def prep_inputs(inp, W_iv, b_iv, W_out, b_out, thresh, k_m, asc_amp, asc_r,
                asc_k, t_steps=T):
    inp = np.asarray(inp, np.float32)
    W_iv = np.asarray(W_iv, np.float32)
    b_iv = np.asarray(b_iv, np.float32)
    W_out = np.asarray(W_out, np.float32)
    b_out = np.asarray(b_out, np.float32)
    thresh = np.asarray(thresh, np.float32).reshape(HID)
    k_m = np.asarray(k_m, np.float32).reshape(HID)
    asc_amp = np.asarray(asc_amp, np.float32).reshape(A, HID)
    asc_r = np.asarray(asc_r, np.float32).reshape(A, HID)
    asc_k = np.asarray(asc_k, np.float32).reshape(A, HID)

    W_ix = W_iv[:, :IN]          # (HID, IN)
    W_hh = W_iv[:, IN:]          # (HID, HID)
    dec = np.exp(np.float32(-DT) * asc_k).astype(np.float32)   # (A, HID)
    km1 = (1.0 - np.float32(DT) * k_m).astype(np.float32)      # (HID,)

    # xT: (IN, T, B) bf16  — same for all cores
    xT = np.ascontiguousarray(inp[:, :t_steps, :].transpose(2, 1, 0))

    in_maps = []
    for c in range(NC_N):
        hs = slice(c * HL, (c + 1) * HL)   # own h rows (global order!)
        # wix[p_k, ki, ho, p_m] = W_ix[c*HL + ho*128 + p_m, ki*128 + p_k]
        wix = W_ix[hs].reshape(HO, 128, NKI, 128).transpose(3, 2, 0, 1)
        whh = W_hh[hs].reshape(HO, 128, NK, 128).transpose(3, 2, 0, 1)
        os_ = slice(c * OL, (c + 1) * OL)
        wo = W_out[os_].reshape(OL, NK, 128).transpose(2, 1, 0)
        shp = lambda x: x[hs].reshape(HO, 128).T.copy()           # [128, HO]
        shpb = lambda x: np.repeat(
            x[hs].reshape(HO, 128).T[:, :, None], B, axis=2)      # [128,HO,B]
        m = {
            "xT": xT,
            "wix": np.ascontiguousarray(wix, np.float32),
            "whh": np.ascontiguousarray(whh, np.float32),
            "wout": np.ascontiguousarray(wo, np.float32),
            "rcol": np.stack([shp(asc_r[a]) for a in range(A)], axis=1).copy(),
            "deccol": np.stack([shp(dec[a]) for a in range(A)], axis=1).copy(),
            "ampt": np.stack([shpb(asc_amp[a]) for a in range(A)], axis=1).copy(),
            "km1t": shpb(km1),
            "negth": shp(-thresh),
            "bivt": shpb(b_iv),
            "boutcol": b_out[os_].reshape(OL, 1).copy(),
        }
        in_maps.append(m)
    return in_maps


def assemble_output(results, t_steps=T):
    # per-core out: [t_steps, OL, B] -> full (B, T, OUT)
    full = np.empty((B, t_steps, OUT), np.float32)
    for c, r in enumerate(results):
        o = r["out"]                      # (t, OL, B)
        full[:, :, c * OL:(c + 1) * OL] = o.transpose(2, 0, 1)
    return full


_CACHE = {}


def _get_nc(t_steps=T):
    if t_steps not in _CACHE:
        _CACHE[t_steps] = build(t_steps)
    return _CACHE[t_steps]


def kernel(inp, W_iv, b_iv, W_out, b_out, thresh, k_m, asc_amp, asc_r, asc_k,
           t_steps=T):
    nc = _get_nc(t_steps)
    in_maps = prep_inputs(inp, W_iv, b_iv, W_out, b_out, thresh, k_m,
                          asc_amp, asc_r, asc_k, t_steps)
    res = bass_utils.run_bass_kernel_spmd(
        nc, in_maps, core_ids=list(range(NC_N)), trace=False)
    return assemble_output(res.results, t_steps)


# revision 31
# speedup vs baseline: 1.1042x; 1.1042x over previous
"""Trainium2 Bass kernel for the BNN/GLIF recurrent network (nn_BNNFC).

Strategy: 8-way tensor parallelism over the hidden dimension H=2048
(256 rows per core). The recurrence over T=512 steps is sequential; each
step does, per core:
  - syn psum = W_ix_shard @ x_t  +  W_hh_shard @ firing_full   (PE, bf16)
  - GLIF elementwise updates (DVE/ACT, f32 state)
  - firing shard -> AllGather across 8 cores -> full firing (h-major)
  - out_t[:, o_shard] = W_out_shard @ firing_full (PE) -> DRAM
Host side shards/transposes inputs and assembles the output.

Layouts (per core, h_local = 256 = 2 m-tiles of 128):
  global h = core*256 + ho*128 + p   (ho in {0,1}, p in [0,128))
  state tiles: [128(p), 2(ho), 64(b)] f32
  firing_full SBUF: [128(p), 8(core), 2(ho), 64(b)] bf16  (16 k-tiles)
"""
import sys, os, time
sys.path.insert(0, "/opt/trn_rl_repo")
import numpy as np

import concourse.bass as bass
import concourse.mybir as mybir
import concourse.tile as tile
from concourse import bacc
from concourse import bass_utils
from concourse.instruction_name_ordered_set import InstructionNameOrderedSet


def _names(*insts):
    s = InstructionNameOrderedSet()
    for i in insts:
        s.add(i.ins.name)
    return s


F32 = mybir.dt.float32
BF16 = mybir.dt.bfloat16

IN, HID, OUT, A = 512, 2048, 512, 2
B, T = 64, 512
DT = 0.05
NC_N = 8            # cores
HL = HID // NC_N    # 256 h rows per core
HO = HL // 128      # 2 m-tiles
OL = OUT // NC_N    # 64 out features per core
NK = HID // 128     # 16 k-tiles over full H
NKI = IN // 128     # 4 k-tiles over input dim


BL = B // NC_N      # 8 batch per core in rep mode
NMO = HID // 128    # 16 m-tiles over full H
NOO = OUT // 128    # 4 out m-tiles


def build_rep(t_steps=T):
    """Replicated weights + batch data-parallel: no collectives at all.
    Every core holds full f32 W_hh/W_ix (LDW-bound, col-tiled) and full
    bf16 W_out (no feedback -> bf16 safe), and runs the whole recurrence
    for its 8-sample batch shard."""
    nc = bacc.Bacc("TRN2", target_bir_lowering=False, debug=False,
                   num_devices=NC_N)
    xT_d = nc.dram_tensor("xT", [t_steps, 128, NKI, BL], F32,
                          kind="ExternalInput")
    wix_d = nc.dram_tensor("wix", [128, NKI, NMO, 128], F32,
                           kind="ExternalInput")
    whh_d = nc.dram_tensor("whh", [128, NK, NMO, 128], F32,
                           kind="ExternalInput")
    wout_d = nc.dram_tensor("wout", [128, NK, NOO, 128], BF16,
                            kind="ExternalInput")
    # full-H broadcast param tiles [128, NMO, BL]
    pnames = ["r1t", "r2t", "amp1t", "amp2t", "dec1t", "dec2t", "km1t",
              "negtht", "bivt", "boutt"]
    pds = {n: nc.dram_tensor(n, [128, NOO if n == "boutt" else NMO, BL],
                             F32, kind="ExternalInput") for n in pnames}
    out_d = nc.dram_tensor("out", [t_steps, 128, NOO, BL], F32,
                           kind="ExternalOutput")

    with tile.TileContext(nc) as tc:
        with (
            tc.tile_pool(name="static", bufs=1) as sp,
            tc.tile_pool(name="state", bufs=1) as st,
            tc.tile_pool(name="fire", bufs=2) as fp,
            tc.tile_pool(name="xin", bufs=3) as xp,
            tc.tile_pool(name="tmp", bufs=2) as tp,
            tc.tile_pool(name="outs", bufs=2) as op_,
            tc.tile_pool(name="psyn", bufs=2, space="PSUM") as pps,
            tc.tile_pool(name="pout", bufs=2, space="PSUM") as ppo,
        ):
            wix = sp.tile([128, NKI, NMO, 128], F32)
            whh = sp.tile([128, NK, NMO, 128], F32)
            wout = sp.tile([128, NK, NOO, 128], BF16)
            nc.sync.dma_start(wix[:], wix_d[:])
            nc.sync.dma_start(whh[:], whh_d[:])
            nc.sync.dma_start(wout[:], wout_d[:])
            pt = {}
            for n in pnames:
                pt[n] = sp.tile([128, NOO if n == "boutt" else NMO, BL],
                                F32, name=n, tag=n)
                nc.sync.dma_start(pt[n][:], pds[n][:])

            v = st.tile([128, NMO, BL], F32)
            a1 = st.tile([128, NMO, BL], F32)
            a2 = st.tile([128, NMO, BL], F32)
            nc.vector.memset(v[:], 0.0)
            nc.vector.memset(a1[:], 0.0)
            nc.vector.memset(a2[:], 0.0)
            fire = fp.tile([128, NMO, BL], F32, tag="fire")
            nc.gpsimd.memset(fire[:], 0.0)

            for t in range(t_steps):
                xt = xp.tile([128, NKI, BL], F32, tag="xt")
                nc.sync.dma_start(xt[:], xT_d[t])

                psyn = pps.tile([128, NMO, BL], F32, tag="psyn")
                for mo in range(NMO):
                    for hf in range(2):
                        sl = slice(hf * 64, (hf + 1) * 64)
                        for ki in range(NKI):
                            nc.tensor.matmul(
                                psyn[sl, mo, :], wix[:, ki, mo, sl],
                                xt[:, ki, :], start=(ki == 0), stop=False,
                                tile_position=(0, hf * 64),
                                skip_group_check=True)
                        for kidx in range(NK):
                            nc.tensor.matmul(
                                psyn[sl, mo, :], whh[:, kidx, mo, sl],
                                fire[:, kidx, :], start=False,
                                stop=(kidx == NK - 1),
                                tile_position=(0, hf * 64),
                                skip_group_check=True)

                # GLIF, full-tile [128, NMO*BL] f32 with broadcast params
                u1 = tp.tile([128, NMO, BL], F32, tag="u1")
                u2 = tp.tile([128, NMO, BL], F32, tag="u2")
                kv = tp.tile([128, NMO, BL], F32, tag="kv")
                fv = tp.tile([128, NMO, BL], F32, tag="fv")
                pre = tp.tile([128, NMO, BL], F32, tag="pre")
                sg = tp.tile([128, NMO, BL], F32, tag="sg")
                nc.vector.tensor_mul(u1[:], a1[:], pt["r1t"][:])
                nc.vector.tensor_add(u1[:], u1[:], pt["amp1t"][:])
                nc.vector.tensor_mul(u2[:], a2[:], pt["r2t"][:])
                nc.vector.tensor_add(u2[:], u2[:], pt["amp2t"][:])
                nc.vector.tensor_mul(u1[:], u1[:], fire[:])
                nc.vector.tensor_mul(u2[:], u2[:], fire[:])
                nc.vector.tensor_mul(a1[:], a1[:], pt["dec1t"][:])
                nc.vector.tensor_add(a1[:], a1[:], u1[:])
                nc.vector.tensor_mul(a2[:], a2[:], pt["dec2t"][:])
                nc.vector.tensor_add(a2[:], a2[:], u2[:])
                nc.vector.tensor_mul(kv[:], v[:], pt["km1t"][:])
                nc.vector.tensor_mul(fv[:], v[:], fire[:])
                nc.vector.tensor_add(pre[:], a1[:], a2[:])
                nc.vector.tensor_add(pre[:], pre[:], pt["bivt"][:])
                nc.vector.tensor_add(pre[:], pre[:], kv[:])
                nc.vector.tensor_tensor(
                    pre[:], pre[:], fv[:], mybir.AluOpType.subtract)
                nc.vector.tensor_add(v[:], psyn[:], pre[:])
                fire_new = fp.tile([128, NMO, BL], F32, tag="fire")
                nc.vector.tensor_add(sg[:], v[:], pt["negtht"][:])
                nc.scalar.activation(
                    fire_new[:], sg[:],
                    mybir.ActivationFunctionType.Sigmoid,
                    bias=0.0, scale=1.0)
                fb = fp.tile([128, NMO, BL], BF16, tag="fireb")
                nc.vector.tensor_copy(fb[:], fire_new[:])

                pout = ppo.tile([128, NOO, BL], F32, tag="pout")
                for mo in range(NOO):
                    for kidx in range(NK):
                        nc.tensor.matmul(
                            pout[:, mo, :], wout[:, kidx, mo, :],
                            fb[:, kidx, :],
                            start=(kidx == 0), stop=(kidx == NK - 1),
                            skip_group_check=True)
                osb = op_.tile([128, NOO, BL], F32, tag="osb")
                nc.vector.tensor_add(osb[:], pout[:], pt["boutt"][:])
                nc.sync.dma_start(out_d[t], osb[:])
                fire = fire_new

    nc.compile()
    return nc


def prep_inputs_rep(inp, W_iv, b_iv, W_out, b_out, thresh, k_m, asc_amp,
                    asc_r, asc_k, t_steps=T):
    inp = np.asarray(inp, np.float32)
    W_iv = np.asarray(W_iv, np.float32)
    b_iv = np.asarray(b_iv, np.float32).reshape(HID)
    W_out = np.asarray(W_out, np.float32)
    b_out = np.asarray(b_out, np.float32).reshape(OUT)
    thresh = np.asarray(thresh, np.float32).reshape(HID)
    k_m = np.asarray(k_m, np.float32).reshape(HID)
    asc_amp = np.asarray(asc_amp, np.float32).reshape(A, HID)
    asc_r = np.asarray(asc_r, np.float32).reshape(A, HID)
    asc_k = np.asarray(asc_k, np.float32).reshape(A, HID)
    W_ix = W_iv[:, :IN]
    W_hh = W_iv[:, IN:]
    dec = np.exp(np.float32(-DT) * asc_k).astype(np.float32)
    km1 = (1.0 - np.float32(DT) * k_m).astype(np.float32)
    import ml_dtypes
    # lhsT layouts: [p_k, kt, mo, p_m]  (h = mo*128 + p_m, k = kt*128 + p_k)
    wix = np.ascontiguousarray(
        W_ix.reshape(NMO, 128, NKI, 128).transpose(3, 2, 0, 1), np.float32)
    whh = np.ascontiguousarray(
        W_hh.reshape(NMO, 128, NK, 128).transpose(3, 2, 0, 1), np.float32)
    wout = np.ascontiguousarray(
        W_out.reshape(NOO, 128, NK, 128).transpose(3, 2, 0, 1)).astype(
            ml_dtypes.bfloat16)
    bch = lambda x, bl: np.repeat(           # [H]->[128, NMO, BL]
        x.reshape(-1, 128).T[:, :, None], bl, axis=2).astype(np.float32)
    shared = {
        "wix": wix, "whh": whh, "wout": wout,
        "r1t": bch(asc_r[0], BL), "r2t": bch(asc_r[1], BL),
        "amp1t": bch(asc_amp[0], BL), "amp2t": bch(asc_amp[1], BL),
        "dec1t": bch(dec[0], BL), "dec2t": bch(dec[1], BL),
        "km1t": bch(km1, BL), "negtht": bch(-thresh, BL),
        "bivt": bch(b_iv, BL), "boutt": bch(b_out, BL),
    }
    in_maps = []
    for c in range(NC_N):
        bs = slice(c * BL, (c + 1) * BL)
        # xT[t, p, ki, b] = inp[b, t, ki*128+p]
        xT = np.ascontiguousarray(
            inp[bs, :t_steps, :].transpose(1, 2, 0).reshape(
                t_steps, NKI, 128, BL).transpose(0, 2, 1, 3), np.float32)
        in_maps.append({**shared, "xT": xT})
    return in_maps


def assemble_output_rep(results, t_steps=T):
    full = np.empty((B, t_steps, OUT), np.float32)
    for c, r in enumerate(results):
        o = r["out"]                       # [t, 128, NOO, BL]
        # o[t, p, mo, b] -> full[c*BL+b, t, mo*128+p]
        full[c * BL:(c + 1) * BL] = o.transpose(3, 0, 2, 1).reshape(
            BL, t_steps, OUT)
    return full


def build(t_steps=T, mode='full'):
    nc = bacc.Bacc("TRN2", target_bir_lowering=False, debug=False,
                   num_devices=NC_N, enable_partition_id=(mode == 'rdma'))

    # ---- external inputs (per-core values supplied via in_maps) ----
    xT_d = nc.dram_tensor("xT", [IN, t_steps, B], F32, kind="ExternalInput")
    wix_d = nc.dram_tensor("wix", [128, NKI, HO, 128], F32, kind="ExternalInput")
    whh_d = nc.dram_tensor("whh", [128, NK, HO, 128], F32, kind="ExternalInput")
    wout_d = nc.dram_tensor("wout", [128, NK, OL], F32, kind="ExternalInput")
    # per-partition param columns [128, HO] and broadcast tiles [128, HO, B]
    rcol_d = nc.dram_tensor("rcol", [128, A, HO], F32, kind="ExternalInput")
    deccol_d = nc.dram_tensor("deccol", [128, A, HO], F32, kind="ExternalInput")
    ampt_d = nc.dram_tensor("ampt", [128, A, HO, B], F32, kind="ExternalInput")
    km1t_d = nc.dram_tensor("km1t", [128, HO, B], F32, kind="ExternalInput")
    negth_d = nc.dram_tensor("negth", [128, HO], F32, kind="ExternalInput")
    bivt_d = nc.dram_tensor("bivt", [128, HO, B], F32, kind="ExternalInput")
    boutcol_d = nc.dram_tensor("boutcol", [OL, 1], F32, kind="ExternalInput")

    out_d = nc.dram_tensor("out", [t_steps, OL, B], F32, kind="ExternalOutput")

    with tile.TileContext(nc) as tc:
        with (
            tc.tile_pool(name="static", bufs=1) as sp,
            tc.tile_pool(name="state", bufs=1) as st,
            tc.tile_pool(name="fire", bufs=2) as fp,
            tc.tile_pool(name="send", bufs=2) as sd,
            tc.tile_pool(name="xin", bufs=3) as xp,
            tc.tile_pool(name="tmp", bufs=2) as tp,
            tc.tile_pool(name="outs", bufs=2) as op_,
            tc.tile_pool(name="psyn", bufs=2, space="PSUM") as pps,
            tc.tile_pool(name="pout", bufs=2, space="PSUM") as ppo,
            tc.tile_pool(name="dram", bufs=2, space="DRAM") as dp,
        ):
            # ---- load static weights/params into SBUF ----
            wix = sp.tile([128, NKI, HO, 128], F32)
            whh = sp.tile([128, NK, HO, 128], F32)
            wout = sp.tile([128, NK, OL], F32)
            rcol = sp.tile([128, A, HO], F32)
            deccol = sp.tile([128, A, HO], F32)
            ampt = sp.tile([128, A, HO, B], F32)
            km1t = sp.tile([128, HO, B], F32)
            negth = sp.tile([128, HO], F32)
            bivt = sp.tile([128, HO, B], F32)
            boutcol = sp.tile([OL, 1], F32)
            nc.sync.dma_start(wix[:], wix_d[:])
            nc.sync.dma_start(whh[:], whh_d[:])
            nc.sync.dma_start(wout[:], wout_d[:])
            nc.sync.dma_start(rcol[:], rcol_d[:])
            nc.sync.dma_start(deccol[:], deccol_d[:])
            nc.sync.dma_start(ampt[:], ampt_d[:])
            nc.sync.dma_start(km1t[:], km1t_d[:])
            nc.sync.dma_start(negth[:], negth_d[:])
            nc.sync.dma_start(bivt[:], bivt_d[:])
            nc.sync.dma_start(boutcol[:], boutcol_d[:])

            # ---- persistent state (f32), zero-init ----
            v = st.tile([128, HO, B], F32)
            a1 = st.tile([128, HO, B], F32)
            a2 = st.tile([128, HO, B], F32)
            fire32 = st.tile([128, HO, B], F32)   # own shard firing f32
            nc.vector.memset(v[:], 0.0)
            nc.vector.memset(a1[:], 0.0)
            nc.vector.memset(a2[:], 0.0)
            nc.vector.memset(fire32[:], 0.0)

            fire_prev = fp.tile([128, NC_N, HO, B], F32, tag="fire")
            nc.gpsimd.memset(fire_prev[:], 0.0)

            if mode == 'rdma':
                rsem = nc.alloc_semaphore("rsem")
                lsem = nc.alloc_semaphore("lsem")
                wkeep = nc.alloc_semaphore("wkeep")
                pid = nc.gpsimd.partition_id()
                nc.gpsimd.sem_clear(rsem)
                nc.gpsimd.sem_clear(lsem)
                # cross-core startup barrier: nobody broadcasts until every
                # core has cleared its semaphores (stale values persist
                # across executions of the same loaded NEFF).
                bar_i = dp.tile([1, 4], F32, tag="bar_i")
                bar_o = dp.tile([NC_N, 1, 4], F32, tag="bar_o")
                bar_inst = nc.gpsimd.collective_compute(
                    "AllGather", mybir.AluOpType.bypass,
                    ins=[bar_i[:].opt()], outs=[bar_o[:].opt()],
                    replica_groups=[list(range(NC_N))])
                pout_last = None
                prev_sig = None
                wait_fixups = []  # (inst_name, sem, value): bump post-compile

            for t in range(t_steps):
                # -- prefetch x_t (bf16 k-tiles) --
                xt = xp.tile([128, NKI, B], F32, tag="xt")
                nc.sync.dma_start(
                    xt[:],
                    xT_d.ap().rearrange("(ki p) tt b -> p ki tt b",
                                        p=128)[:, :, t, :])

                # -- syn matmuls into psum [128, HO, B] --
                # col-tiled matmul pairs: two 64-column stationary loads
                # stream via separate XBUSes, halving the LDWEIGHTS-bound
                # W traversal (LDW cost is per-column).
                psyn = pps.tile([128, HO, B], F32, tag="psyn")
                for ho in range(HO):
                    for ki in range(NKI):
                        for hf in range(2):
                            mm = nc.tensor.matmul(
                                psyn[hf * 64:(hf + 1) * 64, ho, :],
                                wix[:, ki, ho, hf * 64:(hf + 1) * 64],
                                xt[:, ki, :],
                                start=(ki == 0), stop=False,
                                tile_position=(0, hf * 64),
                                skip_group_check=True)
                            if (ki == 0 and hf == 0 and mode == 'rdma'
                                    and pout_last is not None):
                                mm.ins.add_nosync_dependencies_from(
                                    _names(pout_last))
                    for c in range(NC_N):
                        for ho2 in range(HO):
                            kidx = c * HO + ho2
                            last = (c == NC_N - 1 and ho2 == HO - 1)
                            for hf in range(2):
                                mm = nc.tensor.matmul(
                                    psyn[hf * 64:(hf + 1) * 64, ho, :],
                                    whh[:, kidx, ho, hf * 64:(hf + 1) * 64],
                                    fire_prev[:, c, ho2, :],
                                    start=False, stop=last,
                                    tile_position=(0, hf * 64),
                                    skip_group_check=True)
                            syn_last = mm

                # -- GLIF elementwise --
                # IEEE-faithful to the reference (inf/NaN propagation must
                # match: e.g. 0*inf=NaN in km1*v - f*v, so no factoring).
                # off-critical-path pieces (depend only on prev state):
                if mode == 'noglif':
                    send = sd.tile([128, HO, B], F32, tag="send")
                    for ho in range(HO):
                        nc.scalar.activation(
                            send[:, ho, :], psyn[:, ho, :],
                            mybir.ActivationFunctionType.Sigmoid,
                            bias=negth[:, ho:ho + 1], scale=1.0)
                    fire_new = fire_prev
                    pout = ppo.tile([OL, B], F32, tag="pout")
                    for kidx in range(NK):
                        nc.tensor.matmul(
                            pout[:], wout[:, kidx, :],
                            fire_new[:, kidx // HO, kidx % HO, :],
                            start=(kidx == 0), stop=(kidx == NK - 1))
                    osb = op_.tile([OL, B], F32, tag="osb")
                    nc.scalar.activation(
                        osb[:], pout[:], mybir.ActivationFunctionType.Identity,
                        bias=boutcol[:], scale=1.0)
                    nc.sync.dma_start(out_d[t], osb[:])
                    continue
                u1 = tp.tile([128, HO, B], F32, tag="u1")
                u2 = tp.tile([128, HO, B], F32, tag="u2")
                kv = tp.tile([128, HO, B], F32, tag="kv")
                fv = tp.tile([128, HO, B], F32, tag="fv")
                pre = tp.tile([128, HO, B], F32, tag="pre")
                for ho in range(HO):
                    # u_k = r_k * a_k + amp_k
                    nc.vector.scalar_tensor_tensor(
                        u1[:, ho, :], a1[:, ho, :], rcol[:, 0, ho:ho + 1],
                        ampt[:, 0, ho, :], mybir.AluOpType.mult,
                        mybir.AluOpType.add)
                    nc.vector.scalar_tensor_tensor(
                        u2[:, ho, :], a2[:, ho, :], rcol[:, 1, ho:ho + 1],
                        ampt[:, 1, ho, :], mybir.AluOpType.mult,
                        mybir.AluOpType.add)
                # m_k = f * u_k ; a_k' = a_k*dec_k + m_k
                nc.vector.tensor_mul(u1[:], u1[:], fire32[:])
                nc.vector.tensor_mul(u2[:], u2[:], fire32[:])
                for ho in range(HO):
                    nc.vector.scalar_tensor_tensor(
                        a1[:, ho, :], a1[:, ho, :], deccol[:, 0, ho:ho + 1],
                        u1[:, ho, :], mybir.AluOpType.mult, mybir.AluOpType.add)
                    nc.vector.scalar_tensor_tensor(
                        a2[:, ho, :], a2[:, ho, :], deccol[:, 1, ho:ho + 1],
                        u2[:, ho, :], mybir.AluOpType.mult, mybir.AluOpType.add)
                # pre = a1' + a2' + b_iv + km1*v - f*v  (unfactored)
                nc.vector.tensor_mul(kv[:], v[:], km1t[:])
                nc.vector.tensor_mul(fv[:], v[:], fire32[:])
                nc.vector.tensor_add(pre[:], a1[:], a2[:])
                nc.vector.tensor_add(pre[:], pre[:], bivt[:])
                nc.vector.tensor_add(pre[:], pre[:], kv[:])
                nc.vector.tensor_tensor(
                    pre[:], pre[:], fv[:], mybir.AluOpType.subtract)
                # critical: v = psum + pre ; f = sigmoid(v - thresh)
                nc.vector.tensor_add(v[:], psyn[:], pre[:])
                send = sd.tile([128, HO, B], F32, tag="send")
                wact = None
                if mode == 'rdma' and t >= 2:
                    wact = nc.scalar.wait_ge(lsem, 0).then_inc(wkeep, 1)
                    wait_fixups.append((wact.ins.name, lsem, 16 * (t - 1)))
                    if prev_sig is not None:
                        wact.ins.add_nosync_dependencies_from(_names(prev_sig))
                for ho in range(HO):
                    si = nc.scalar.activation(
                        send[:, ho, :], v[:, ho, :],
                        mybir.ActivationFunctionType.Sigmoid,
                        bias=negth[:, ho:ho + 1], scale=1.0)
                    if wact is not None:
                        si.ins.add_nosync_dependencies_from(_names(wact))
                    prev_sig = si
                # fire32 = local f32 firing for next step's state math
                nc.vector.tensor_copy(fire32[:], send[:])

                # -- allgather firing across cores --
                if mode == 'rdma':
                    fire_new = fp.tile([128, NC_N, HO, B], F32, tag="fire")
                    prep = nc.gpsimd.remote_dma_broadcast(
                        out_ap=fire_new[:, bass.ds(pid, 1), :, :],
                        in_ap=send[:],
                        remote_sem=rsem, local_sem=lsem,
                        rdests=[(0, k) for k in range(NC_N)])
                    if t == 0:
                        prep.ins.add_nosync_dependencies_from(
                            _names(bar_inst))
                    trig = nc.gpsimd.trigger_dma(count=None)
                    wpe = nc.tensor.wait_ge(rsem, 0).then_inc(wkeep, 1)
                    wait_fixups.append((wpe.ins.name, rsem, 16 * (t + 1)))
                    wpe.ins.add_nosync_dependencies_from(_names(syn_last))
                elif mode != 'noag':
                    # ONE AllGather per step: each collective has a ~13.5 us
                    # fixed cost here (measured: splitting into two 32KB AGs
                    # costs +13.5 us/step), so minimize collective count.
                    in_b = dp.tile([128, HO, B], F32, tag="agin")
                    out_b = dp.tile([NC_N, 128, HO, B], F32, tag="agout")
                    nc.sync.dma_start(in_b[:], send[:])
                    nc.gpsimd.collective_compute(
                        "AllGather", mybir.AluOpType.bypass,
                        ins=[in_b[:].opt()], outs=[out_b[:].opt()],
                        replica_groups=[list(range(NC_N))])
                    fire_new = fp.tile([128, NC_N, HO, B], F32, tag="fire")
                    rb = out_b[:].rearrange("c p ho b -> p c ho b")
                    for q in range(4):
                        nc.sync.dma_start(
                            fire_new[:, 2 * q:2 * q + 2, :, :],
                            rb[:, 2 * q:2 * q + 2, :, :])
                else:
                    fire_new = fire_prev

                # -- output projection for this step (uses gathered fire_new) --
                pout = ppo.tile([OL, B], F32, tag="pout")
                for kidx in range(NK):
                    mm = nc.tensor.matmul(
                        pout[:], wout[:, kidx, :],
                        fire_new[:, kidx // HO, kidx % HO, :],
                        start=(kidx == 0), stop=(kidx == NK - 1))
                    if mode == 'rdma':
                        if kidx == 0:
                            mm.ins.add_nosync_dependencies_from(_names(wpe))
                        if kidx == NK - 1:
                            pout_last = mm
                osb = op_.tile([OL, B], F32, tag="osb")
                nc.scalar.activation(
                    osb[:], pout[:], mybir.ActivationFunctionType.Identity,
                    bias=boutcol[:], scale=1.0)
                nc.sync.dma_start(out_d[t], osb[:])

                fire_prev = fire_new

            if mode == 'rdma':
                wq1 = nc.gpsimd.wait_ge(rsem, 0).then_inc(wkeep, 1)
                wq2 = nc.gpsimd.wait_ge(lsem, 0).then_inc(wkeep, 1)
                wq1.ins.add_nosync_dependencies_from(_names(trig))
                wq2.ins.add_nosync_dependencies_from(_names(wq1))
                wait_fixups.append((wq1.ins.name, rsem, 16 * t_steps))
                wait_fixups.append((wq2.ins.name, lsem, 16 * t_steps))

    nc.compile()
    if mode == 'rdma':
        fix = {n: (sem, val) for n, sem, val in wait_fixups}
        done = 0
        for f in nc.m.functions:
            for bb in f.blocks:
                for i in bb.instructions:
                    if i.name in fix:
                        sem, val = fix[i.name]
                        w = mybir.SyncWait(
                            sync_type='semaphore', id=sem.num,
                            ant_name=sem.name, wait_mode='sem-ge-imm',
                            wait_value=val)
                        si = i.sync_info
                        keep = [x for x in (si.on_wait if si else [])
                                if not (x.id == sem.num)]
                        i.sync_info = mybir.SyncInfo(
                            on_wait=keep + [w],
                            on_update=list(si.on_update) if si else [])
                        done += 1
        assert done == len(fix), f"wait fixups applied {done}/{len(fix)}"
    return nc


# ---------------- host-side data prep ----------------
def prep_inputs(inp, W_iv, b_iv, W_out, b_out, thresh, k_m, asc_amp, asc_r,
                asc_k, t_steps=T):
    inp = np.asarray(inp, np.float32)
    W_iv = np.asarray(W_iv, np.float32)
    b_iv = np.asarray(b_iv, np.float32)
    W_out = np.asarray(W_out, np.float32)
    b_out = np.asarray(b_out, np.float32)
    thresh = np.asarray(thresh, np.float32).reshape(HID)
    k_m = np.asarray(k_m, np.float32).reshape(HID)
    asc_amp = np.asarray(asc_amp, np.float32).reshape(A, HID)
    asc_r = np.asarray(asc_r, np.float32).reshape(A, HID)
    asc_k = np.asarray(asc_k, np.float32).reshape(A, HID)

    W_ix = W_iv[:, :IN]          # (HID, IN)
    W_hh = W_iv[:, IN:]          # (HID, HID)
    dec = np.exp(np.float32(-DT) * asc_k).astype(np.float32)   # (A, HID)
    km1 = (1.0 - np.float32(DT) * k_m).astype(np.float32)      # (HID,)

    # xT: (IN, T, B) bf16  — same for all cores
    xT = np.ascontiguousarray(inp[:, :t_steps, :].transpose(2, 1, 0))

    in_maps = []
    for c in range(NC_N):
        hs = slice(c * HL, (c + 1) * HL)   # own h rows (global order!)
        # wix[p_k, ki, ho, p_m] = W_ix[c*HL + ho*128 + p_m, ki*128 + p_k]
        wix = W_ix[hs].reshape(HO, 128, NKI, 128).transpose(3, 2, 0, 1)
        whh = W_hh[hs].reshape(HO, 128, NK, 128).transpose(3, 2, 0, 1)
        os_ = slice(c * OL, (c + 1) * OL)
        wo = W_out[os_].reshape(OL, NK, 128).transpose(2, 1, 0)
        shp = lambda x: x[hs].reshape(HO, 128).T.copy()           # [128, HO]
        shpb = lambda x: np.repeat(
            x[hs].reshape(HO, 128).T[:, :, None], B, axis=2)      # [128,HO,B]
        m = {
            "xT": xT,
            "wix": np.ascontiguousarray(wix, np.float32),
            "whh": np.ascontiguousarray(whh, np.float32),
            "wout": np.ascontiguousarray(wo, np.float32),
            "rcol": np.stack([shp(asc_r[a]) for a in range(A)], axis=1).copy(),
            "deccol": np.stack([shp(dec[a]) for a in range(A)], axis=1).copy(),
            "ampt": np.stack([shpb(asc_amp[a]) for a in range(A)], axis=1).copy(),
            "km1t": shpb(km1),
            "negth": shp(-thresh),
            "bivt": shpb(b_iv),
            "boutcol": b_out[os_].reshape(OL, 1).copy(),
        }
        in_maps.append(m)
    return in_maps


def assemble_output(results, t_steps=T):
    # per-core out: [t_steps, OL, B] -> full (B, T, OUT)
    full = np.empty((B, t_steps, OUT), np.float32)
    for c, r in enumerate(results):
        o = r["out"]                      # (t, OL, B)
        full[:, :, c * OL:(c + 1) * OL] = o.transpose(2, 0, 1)
    return full


_CACHE = {}


def _get_nc(t_steps=T):
    if t_steps not in _CACHE:
        _CACHE[t_steps] = build(t_steps)
    return _CACHE[t_steps]


def kernel(inp, W_iv, b_iv, W_out, b_out, thresh, k_m, asc_amp, asc_r, asc_k,
           t_steps=T):
    nc = _get_nc(t_steps)
    in_maps = prep_inputs(inp, W_iv, b_iv, W_out, b_out, thresh, k_m,
                          asc_amp, asc_r, asc_k, t_steps)
    res = bass_utils.run_bass_kernel_spmd(
        nc, in_maps, core_ids=list(range(NC_N)), trace=False)
    return assemble_output(res.results, t_steps)


# revision 32
# speedup vs baseline: 1.1922x; 1.0796x over previous
"""Trainium2 Bass kernel for the BNN/GLIF recurrent network (nn_BNNFC).

Strategy: 8-way tensor parallelism over the hidden dimension H=2048
(256 rows per core). The recurrence over T=512 steps is sequential; each
step does, per core:
  - syn psum = W_ix_shard @ x_t  +  W_hh_shard @ firing_full   (PE, bf16)
  - GLIF elementwise updates (DVE/ACT, f32 state)
  - firing shard -> AllGather across 8 cores -> full firing (h-major)
  - out_t[:, o_shard] = W_out_shard @ firing_full (PE) -> DRAM
Host side shards/transposes inputs and assembles the output.

Layouts (per core, h_local = 256 = 2 m-tiles of 128):
  global h = core*256 + ho*128 + p   (ho in {0,1}, p in [0,128))
  state tiles: [128(p), 2(ho), 64(b)] f32
  firing_full SBUF: [128(p), 8(core), 2(ho), 64(b)] bf16  (16 k-tiles)
"""
import sys, os, time
sys.path.insert(0, "/opt/trn_rl_repo")
import numpy as np

import concourse.bass as bass
import concourse.mybir as mybir
import concourse.tile as tile
from concourse import bacc
from concourse import bass_utils
from concourse.instruction_name_ordered_set import InstructionNameOrderedSet


def _names(*insts):
    s = InstructionNameOrderedSet()
    for i in insts:
        s.add(i.ins.name)
    return s


F32 = mybir.dt.float32
BF16 = mybir.dt.bfloat16

IN, HID, OUT, A = 512, 2048, 512, 2
B, T = 64, 512
DT = 0.05
NC_N = 8            # cores
HL = HID // NC_N    # 256 h rows per core
HO = HL // 128      # 2 m-tiles
OL = OUT // NC_N    # 64 out features per core
NK = HID // 128     # 16 k-tiles over full H
NKI = IN // 128     # 4 k-tiles over input dim


BL = B // NC_N      # 8 batch per core in rep mode
NMO = HID // 128    # 16 m-tiles over full H
NOO = OUT // 128    # 4 out m-tiles


def build_rep(t_steps=T):
    """Replicated weights + batch data-parallel: no collectives at all.
    Every core holds full f32 W_hh/W_ix (LDW-bound, col-tiled) and full
    bf16 W_out (no feedback -> bf16 safe), and runs the whole recurrence
    for its 8-sample batch shard."""
    nc = bacc.Bacc("TRN2", target_bir_lowering=False, debug=False,
                   num_devices=NC_N)
    xT_d = nc.dram_tensor("xT", [t_steps, 128, NKI, BL], F32,
                          kind="ExternalInput")
    wix_d = nc.dram_tensor("wix", [128, NKI, NMO, 128], F32,
                           kind="ExternalInput")
    whh_d = nc.dram_tensor("whh", [128, NK, NMO, 128], F32,
                           kind="ExternalInput")
    wout_d = nc.dram_tensor("wout", [128, NK, NOO, 128], BF16,
                            kind="ExternalInput")
    # full-H broadcast param tiles [128, NMO, BL]
    pnames = ["r1t", "r2t", "amp1t", "amp2t", "dec1t", "dec2t", "km1t",
              "negtht", "bivt", "boutt"]
    pds = {n: nc.dram_tensor(n, [128, NOO if n == "boutt" else NMO, BL],
                             F32, kind="ExternalInput") for n in pnames}
    out_d = nc.dram_tensor("out", [t_steps, 128, NOO, BL], F32,
                           kind="ExternalOutput")

    with tile.TileContext(nc) as tc:
        with (
            tc.tile_pool(name="static", bufs=1) as sp,
            tc.tile_pool(name="state", bufs=1) as st,
            tc.tile_pool(name="fire", bufs=2) as fp,
            tc.tile_pool(name="xin", bufs=3) as xp,
            tc.tile_pool(name="tmp", bufs=2) as tp,
            tc.tile_pool(name="outs", bufs=2) as op_,
            tc.tile_pool(name="psyn", bufs=2, space="PSUM") as pps,
            tc.tile_pool(name="pout", bufs=2, space="PSUM") as ppo,
        ):
            wix = sp.tile([128, NKI, NMO, 128], F32)
            whh = sp.tile([128, NK, NMO, 128], F32)
            wout = sp.tile([128, NK, NOO, 128], BF16)
            nc.sync.dma_start(wix[:], wix_d[:])
            nc.sync.dma_start(whh[:], whh_d[:])
            nc.sync.dma_start(wout[:], wout_d[:])
            pt = {}
            for n in pnames:
                pt[n] = sp.tile([128, NOO if n == "boutt" else NMO, BL],
                                F32, name=n, tag=n)
                nc.sync.dma_start(pt[n][:], pds[n][:])

            v = st.tile([128, NMO, BL], F32)
            a1 = st.tile([128, NMO, BL], F32)
            a2 = st.tile([128, NMO, BL], F32)
            nc.vector.memset(v[:], 0.0)
            nc.vector.memset(a1[:], 0.0)
            nc.vector.memset(a2[:], 0.0)
            fire = fp.tile([128, NMO, BL], F32, tag="fire")
            nc.gpsimd.memset(fire[:], 0.0)

            for t in range(t_steps):
                xt = xp.tile([128, NKI, BL], F32, tag="xt")
                nc.sync.dma_start(xt[:], xT_d[t])

                psyn = pps.tile([128, NMO, BL], F32, tag="psyn")
                for mo in range(NMO):
                    for hf in range(2):
                        sl = slice(hf * 64, (hf + 1) * 64)
                        for ki in range(NKI):
                            nc.tensor.matmul(
                                psyn[sl, mo, :], wix[:, ki, mo, sl],
                                xt[:, ki, :], start=(ki == 0), stop=False,
                                tile_position=(0, hf * 64),
                                skip_group_check=True)
                        for kidx in range(NK):
                            nc.tensor.matmul(
                                psyn[sl, mo, :], whh[:, kidx, mo, sl],
                                fire[:, kidx, :], start=False,
                                stop=(kidx == NK - 1),
                                tile_position=(0, hf * 64),
                                skip_group_check=True)

                # GLIF, full-tile [128, NMO*BL] f32 with broadcast params
                u1 = tp.tile([128, NMO, BL], F32, tag="u1")
                u2 = tp.tile([128, NMO, BL], F32, tag="u2")
                kv = tp.tile([128, NMO, BL], F32, tag="kv")
                fv = tp.tile([128, NMO, BL], F32, tag="fv")
                pre = tp.tile([128, NMO, BL], F32, tag="pre")
                sg = tp.tile([128, NMO, BL], F32, tag="sg")
                nc.vector.tensor_mul(u1[:], a1[:], pt["r1t"][:])
                nc.vector.tensor_add(u1[:], u1[:], pt["amp1t"][:])
                nc.vector.tensor_mul(u2[:], a2[:], pt["r2t"][:])
                nc.vector.tensor_add(u2[:], u2[:], pt["amp2t"][:])
                nc.vector.tensor_mul(u1[:], u1[:], fire[:])
                nc.vector.tensor_mul(u2[:], u2[:], fire[:])
                nc.vector.tensor_mul(a1[:], a1[:], pt["dec1t"][:])
                nc.vector.tensor_add(a1[:], a1[:], u1[:])
                nc.vector.tensor_mul(a2[:], a2[:], pt["dec2t"][:])
                nc.vector.tensor_add(a2[:], a2[:], u2[:])
                nc.vector.tensor_mul(kv[:], v[:], pt["km1t"][:])
                nc.vector.tensor_mul(fv[:], v[:], fire[:])
                nc.vector.tensor_add(pre[:], a1[:], a2[:])
                nc.vector.tensor_add(pre[:], pre[:], pt["bivt"][:])
                nc.vector.tensor_add(pre[:], pre[:], kv[:])
                nc.vector.tensor_tensor(
                    pre[:], pre[:], fv[:], mybir.AluOpType.subtract)
                nc.vector.tensor_add(v[:], psyn[:], pre[:])
                fire_new = fp.tile([128, NMO, BL], F32, tag="fire")
                nc.vector.tensor_add(sg[:], v[:], pt["negtht"][:])
                nc.scalar.activation(
                    fire_new[:], sg[:],
                    mybir.ActivationFunctionType.Sigmoid,
                    bias=0.0, scale=1.0)
                fb = fp.tile([128, NMO, BL], BF16, tag="fireb")
                nc.vector.tensor_copy(fb[:], fire_new[:])

                pout = ppo.tile([128, NOO, BL], F32, tag="pout")
                for mo in range(NOO):
                    for kidx in range(NK):
                        nc.tensor.matmul(
                            pout[:, mo, :], wout[:, kidx, mo, :],
                            fb[:, kidx, :],
                            start=(kidx == 0), stop=(kidx == NK - 1),
                            skip_group_check=True)
                osb = op_.tile([128, NOO, BL], F32, tag="osb")
                nc.vector.tensor_add(osb[:], pout[:], pt["boutt"][:])
                nc.sync.dma_start(out_d[t], osb[:])
                fire = fire_new

    nc.compile()
    return nc


def prep_inputs_rep(inp, W_iv, b_iv, W_out, b_out, thresh, k_m, asc_amp,
                    asc_r, asc_k, t_steps=T):
    inp = np.asarray(inp, np.float32)
    W_iv = np.asarray(W_iv, np.float32)
    b_iv = np.asarray(b_iv, np.float32).reshape(HID)
    W_out = np.asarray(W_out, np.float32)
    b_out = np.asarray(b_out, np.float32).reshape(OUT)
    thresh = np.asarray(thresh, np.float32).reshape(HID)
    k_m = np.asarray(k_m, np.float32).reshape(HID)
    asc_amp = np.asarray(asc_amp, np.float32).reshape(A, HID)
    asc_r = np.asarray(asc_r, np.float32).reshape(A, HID)
    asc_k = np.asarray(asc_k, np.float32).reshape(A, HID)
    W_ix = W_iv[:, :IN]
    W_hh = W_iv[:, IN:]
    dec = np.exp(np.float32(-DT) * asc_k).astype(np.float32)
    km1 = (1.0 - np.float32(DT) * k_m).astype(np.float32)
    import ml_dtypes
    # lhsT layouts: [p_k, kt, mo, p_m]  (h = mo*128 + p_m, k = kt*128 + p_k)
    wix = np.ascontiguousarray(
        W_ix.reshape(NMO, 128, NKI, 128).transpose(3, 2, 0, 1), np.float32)
    whh = np.ascontiguousarray(
        W_hh.reshape(NMO, 128, NK, 128).transpose(3, 2, 0, 1), np.float32)
    wout = np.ascontiguousarray(
        W_out.reshape(NOO, 128, NK, 128).transpose(3, 2, 0, 1)).astype(
            ml_dtypes.bfloat16)
    bch = lambda x, bl: np.repeat(           # [H]->[128, NMO, BL]
        x.reshape(-1, 128).T[:, :, None], bl, axis=2).astype(np.float32)
    shared = {
        "wix": wix, "whh": whh, "wout": wout,
        "r1t": bch(asc_r[0], BL), "r2t": bch(asc_r[1], BL),
        "amp1t": bch(asc_amp[0], BL), "amp2t": bch(asc_amp[1], BL),
        "dec1t": bch(dec[0], BL), "dec2t": bch(dec[1], BL),
        "km1t": bch(km1, BL), "negtht": bch(-thresh, BL),
        "bivt": bch(b_iv, BL), "boutt": bch(b_out, BL),
    }
    in_maps = []
    for c in range(NC_N):
        bs = slice(c * BL, (c + 1) * BL)
        # xT[t, p, ki, b] = inp[b, t, ki*128+p]
        xT = np.ascontiguousarray(
            inp[bs, :t_steps, :].transpose(1, 2, 0).reshape(
                t_steps, NKI, 128, BL).transpose(0, 2, 1, 3), np.float32)
        in_maps.append({**shared, "xT": xT})
    return in_maps


def assemble_output_rep(results, t_steps=T):
    full = np.empty((B, t_steps, OUT), np.float32)
    for c, r in enumerate(results):
        o = r["out"]                       # [t, 128, NOO, BL]
        # o[t, p, mo, b] -> full[c*BL+b, t, mo*128+p]
        full[c * BL:(c + 1) * BL] = o.transpose(3, 0, 2, 1).reshape(
            BL, t_steps, OUT)
    return full


def build(t_steps=T, mode='full'):
    nc = bacc.Bacc("TRN2", target_bir_lowering=False, debug=False,
                   num_devices=NC_N, enable_partition_id=(mode == 'rdma'))

    # ---- external inputs (per-core values supplied via in_maps) ----
    xT_d = nc.dram_tensor("xT", [IN, t_steps, B], F32, kind="ExternalInput")
    wix_d = nc.dram_tensor("wix", [128, NKI, HO, 128], F32, kind="ExternalInput")
    whh_d = nc.dram_tensor("whh", [128, NK, HO, 128], F32, kind="ExternalInput")
    wout_d = nc.dram_tensor("wout", [128, NK, OL], F32, kind="ExternalInput")
    # per-partition param columns [128, HO] and broadcast tiles [128, HO, B]
    rcol_d = nc.dram_tensor("rcol", [128, A, HO], F32, kind="ExternalInput")
    deccol_d = nc.dram_tensor("deccol", [128, A, HO], F32, kind="ExternalInput")
    ampt_d = nc.dram_tensor("ampt", [128, A, HO, B], F32, kind="ExternalInput")
    km1t_d = nc.dram_tensor("km1t", [128, HO, B], F32, kind="ExternalInput")
    negth_d = nc.dram_tensor("negth", [128, HO], F32, kind="ExternalInput")
    bivt_d = nc.dram_tensor("bivt", [128, HO, B], F32, kind="ExternalInput")
    boutcol_d = nc.dram_tensor("boutcol", [OL, 1], F32, kind="ExternalInput")

    out_d = nc.dram_tensor("out", [t_steps, OL, B], F32, kind="ExternalOutput")

    with tile.TileContext(nc) as tc:
        with (
            tc.tile_pool(name="static", bufs=1) as sp,
            tc.tile_pool(name="state", bufs=1) as st,
            tc.tile_pool(name="fire", bufs=2) as fp,
            tc.tile_pool(name="send", bufs=2) as sd,
            tc.tile_pool(name="xin", bufs=3) as xp,
            tc.tile_pool(name="tmp", bufs=2) as tp,
            tc.tile_pool(name="outs", bufs=2) as op_,
            tc.tile_pool(name="psyn", bufs=2, space="PSUM") as pps,
            tc.tile_pool(name="pout", bufs=2, space="PSUM") as ppo,
            tc.tile_pool(name="dram", bufs=2, space="DRAM") as dp,
        ):
            # ---- load static weights/params into SBUF ----
            wix = sp.tile([128, NKI, HO, 128], F32)
            whh = sp.tile([128, NK, HO, 128], F32)
            wout = sp.tile([128, NK, OL], F32)
            rcol = sp.tile([128, A, HO], F32)
            deccol = sp.tile([128, A, HO], F32)
            ampt = sp.tile([128, A, HO, B], F32)
            km1t = sp.tile([128, HO, B], F32)
            negth = sp.tile([128, HO], F32)
            bivt = sp.tile([128, HO, B], F32)
            boutcol = sp.tile([OL, 1], F32)
            nc.sync.dma_start(wix[:], wix_d[:])
            nc.sync.dma_start(whh[:], whh_d[:])
            nc.sync.dma_start(wout[:], wout_d[:])
            nc.sync.dma_start(rcol[:], rcol_d[:])
            nc.sync.dma_start(deccol[:], deccol_d[:])
            nc.sync.dma_start(ampt[:], ampt_d[:])
            nc.sync.dma_start(km1t[:], km1t_d[:])
            nc.sync.dma_start(negth[:], negth_d[:])
            nc.sync.dma_start(bivt[:], bivt_d[:])
            nc.sync.dma_start(boutcol[:], boutcol_d[:])

            # ---- persistent state (f32), zero-init ----
            v = st.tile([128, HO, B], F32)
            a1 = st.tile([128, HO, B], F32)
            a2 = st.tile([128, HO, B], F32)
            fire32 = st.tile([128, HO, B], F32)   # own shard firing f32
            nc.vector.memset(v[:], 0.0)
            nc.vector.memset(a1[:], 0.0)
            nc.vector.memset(a2[:], 0.0)
            nc.vector.memset(fire32[:], 0.0)

            fire_prev = fp.tile([128, NC_N, HO, B], F32, tag="fire")
            nc.gpsimd.memset(fire_prev[:], 0.0)
            out_pending = None

            def emit_outproj(tt_, ftile):
                pout = ppo.tile([OL, B], F32, tag="pout", name="pout")
                for kidx in range(NK):
                    nc.tensor.matmul(
                        pout[:], wout[:, kidx, :],
                        ftile[:, kidx // HO, kidx % HO, :],
                        start=(kidx == 0), stop=(kidx == NK - 1))
                osb = op_.tile([OL, B], F32, tag="osb", name="osb")
                nc.scalar.activation(
                    osb[:], pout[:], mybir.ActivationFunctionType.Identity,
                    bias=boutcol[:], scale=1.0)
                nc.sync.dma_start(out_d[tt_], osb[:])

            if mode == 'rdma':
                rsem = nc.alloc_semaphore("rsem")
                lsem = nc.alloc_semaphore("lsem")
                wkeep = nc.alloc_semaphore("wkeep")
                pid = nc.gpsimd.partition_id()
                nc.gpsimd.sem_clear(rsem)
                nc.gpsimd.sem_clear(lsem)
                # cross-core startup barrier: nobody broadcasts until every
                # core has cleared its semaphores (stale values persist
                # across executions of the same loaded NEFF).
                bar_i = dp.tile([1, 4], F32, tag="bar_i")
                bar_o = dp.tile([NC_N, 1, 4], F32, tag="bar_o")
                bar_inst = nc.gpsimd.collective_compute(
                    "AllGather", mybir.AluOpType.bypass,
                    ins=[bar_i[:].opt()], outs=[bar_o[:].opt()],
                    replica_groups=[list(range(NC_N))])
                pout_last = None
                prev_sig = None
                wait_fixups = []  # (inst_name, sem, value): bump post-compile

            for t in range(t_steps):
                # -- prefetch x_t (bf16 k-tiles) --
                xt = xp.tile([128, NKI, B], F32, tag="xt")
                nc.sync.dma_start(
                    xt[:],
                    xT_d.ap().rearrange("(ki p) tt b -> p ki tt b",
                                        p=128)[:, :, t, :])

                # -- syn matmuls into psum [128, HO, B] --
                # col-tiled matmul pairs: two 64-column stationary loads
                # stream via separate XBUSes, halving the LDWEIGHTS-bound
                # W traversal (LDW cost is per-column).
                psyn = pps.tile([128, HO, B], F32, tag="psyn")
                for ho in range(HO):
                    for ki in range(NKI):
                        for hf in range(2):
                            mm = nc.tensor.matmul(
                                psyn[hf * 64:(hf + 1) * 64, ho, :],
                                wix[:, ki, ho, hf * 64:(hf + 1) * 64],
                                xt[:, ki, :],
                                start=(ki == 0), stop=False,
                                tile_position=(0, hf * 64),
                                skip_group_check=True)
                            if (ki == 0 and hf == 0 and mode == 'rdma'
                                    and pout_last is not None):
                                mm.ins.add_nosync_dependencies_from(
                                    _names(pout_last))
                    for c in range(NC_N):
                        for ho2 in range(HO):
                            kidx = c * HO + ho2
                            last = (c == NC_N - 1 and ho2 == HO - 1)
                            for hf in range(2):
                                mm = nc.tensor.matmul(
                                    psyn[hf * 64:(hf + 1) * 64, ho, :],
                                    whh[:, kidx, ho, hf * 64:(hf + 1) * 64],
                                    fire_prev[:, c, ho2, :],
                                    start=False, stop=last,
                                    tile_position=(0, hf * 64),
                                    skip_group_check=True)
                            syn_last = mm

                # deferred output projection for the PREVIOUS step: emitted
                # after this step's syn matmuls so the x-projection mms (which
                # don't need the gather) aren't head-of-line blocked on the
                # PE stream behind an AG-dependent out-proj.
                if out_pending is not None:
                    emit_outproj(out_pending, fire_prev)
                    out_pending = None

                # -- GLIF elementwise --
                # IEEE-faithful to the reference (inf/NaN propagation must
                # match: e.g. 0*inf=NaN in km1*v - f*v, so no factoring).
                # off-critical-path pieces (depend only on prev state):
                if mode == 'noglif':
                    send = sd.tile([128, HO, B], F32, tag="send")
                    for ho in range(HO):
                        nc.scalar.activation(
                            send[:, ho, :], psyn[:, ho, :],
                            mybir.ActivationFunctionType.Sigmoid,
                            bias=negth[:, ho:ho + 1], scale=1.0)
                    fire_new = fire_prev
                    pout = ppo.tile([OL, B], F32, tag="pout")
                    for kidx in range(NK):
                        nc.tensor.matmul(
                            pout[:], wout[:, kidx, :],
                            fire_new[:, kidx // HO, kidx % HO, :],
                            start=(kidx == 0), stop=(kidx == NK - 1))
                    osb = op_.tile([OL, B], F32, tag="osb")
                    nc.scalar.activation(
                        osb[:], pout[:], mybir.ActivationFunctionType.Identity,
                        bias=boutcol[:], scale=1.0)
                    nc.sync.dma_start(out_d[t], osb[:])
                    continue
                u1 = tp.tile([128, HO, B], F32, tag="u1")
                u2 = tp.tile([128, HO, B], F32, tag="u2")
                kv = tp.tile([128, HO, B], F32, tag="kv")
                fv = tp.tile([128, HO, B], F32, tag="fv")
                pre = tp.tile([128, HO, B], F32, tag="pre")
                for ho in range(HO):
                    # u_k = r_k * a_k + amp_k
                    nc.vector.scalar_tensor_tensor(
                        u1[:, ho, :], a1[:, ho, :], rcol[:, 0, ho:ho + 1],
                        ampt[:, 0, ho, :], mybir.AluOpType.mult,
                        mybir.AluOpType.add)
                    nc.vector.scalar_tensor_tensor(
                        u2[:, ho, :], a2[:, ho, :], rcol[:, 1, ho:ho + 1],
                        ampt[:, 1, ho, :], mybir.AluOpType.mult,
                        mybir.AluOpType.add)
                # m_k = f * u_k ; a_k' = a_k*dec_k + m_k
                nc.vector.tensor_mul(u1[:], u1[:], fire32[:])
                nc.vector.tensor_mul(u2[:], u2[:], fire32[:])
                for ho in range(HO):
                    nc.vector.scalar_tensor_tensor(
                        a1[:, ho, :], a1[:, ho, :], deccol[:, 0, ho:ho + 1],
                        u1[:, ho, :], mybir.AluOpType.mult, mybir.AluOpType.add)
                    nc.vector.scalar_tensor_tensor(
                        a2[:, ho, :], a2[:, ho, :], deccol[:, 1, ho:ho + 1],
                        u2[:, ho, :], mybir.AluOpType.mult, mybir.AluOpType.add)
                # pre = a1' + a2' + b_iv + km1*v - f*v  (unfactored)
                nc.vector.tensor_mul(kv[:], v[:], km1t[:])
                nc.vector.tensor_mul(fv[:], v[:], fire32[:])
                nc.vector.tensor_add(pre[:], a1[:], a2[:])
                nc.vector.tensor_add(pre[:], pre[:], bivt[:])
                nc.vector.tensor_add(pre[:], pre[:], kv[:])
                nc.vector.tensor_tensor(
                    pre[:], pre[:], fv[:], mybir.AluOpType.subtract)
                # critical: v = psum + pre ; f = sigmoid(v - thresh)
                nc.vector.tensor_add(v[:], psyn[:], pre[:])
                send = sd.tile([128, HO, B], F32, tag="send")
                wact = None
                if mode == 'rdma' and t >= 2:
                    wact = nc.scalar.wait_ge(lsem, 0).then_inc(wkeep, 1)
                    wait_fixups.append((wact.ins.name, lsem, 16 * (t - 1)))
                    if prev_sig is not None:
                        wact.ins.add_nosync_dependencies_from(_names(prev_sig))
                for ho in range(HO):
                    si = nc.scalar.activation(
                        send[:, ho, :], v[:, ho, :],
                        mybir.ActivationFunctionType.Sigmoid,
                        bias=negth[:, ho:ho + 1], scale=1.0)
                    if wact is not None:
                        si.ins.add_nosync_dependencies_from(_names(wact))
                    prev_sig = si
                # fire32 = local f32 firing for next step's state math
                nc.vector.tensor_copy(fire32[:], send[:])

                # -- allgather firing across cores --
                if mode == 'rdma':
                    fire_new = fp.tile([128, NC_N, HO, B], F32, tag="fire")
                    prep = nc.gpsimd.remote_dma_broadcast(
                        out_ap=fire_new[:, bass.ds(pid, 1), :, :],
                        in_ap=send[:],
                        remote_sem=rsem, local_sem=lsem,
                        rdests=[(0, k) for k in range(NC_N)])
                    if t == 0:
                        prep.ins.add_nosync_dependencies_from(
                            _names(bar_inst))
                    trig = nc.gpsimd.trigger_dma(count=None)
                    wpe = nc.tensor.wait_ge(rsem, 0).then_inc(wkeep, 1)
                    wait_fixups.append((wpe.ins.name, rsem, 16 * (t + 1)))
                    wpe.ins.add_nosync_dependencies_from(_names(syn_last))
                elif mode != 'noag':
                    # ONE AllGather per step: each collective has a ~13.5 us
                    # fixed cost here (measured: splitting into two 32KB AGs
                    # costs +13.5 us/step), so minimize collective count.
                    in_b = dp.tile([128, HO, B], F32, tag="agin")
                    out_b = dp.tile([NC_N, 128, HO, B], F32, tag="agout")
                    nc.sync.dma_start(in_b[:], send[:])
                    nc.gpsimd.collective_compute(
                        "AllGather", mybir.AluOpType.bypass,
                        ins=[in_b[:].opt()], outs=[out_b[:].opt()],
                        replica_groups=[list(range(NC_N))])
                    fire_new = fp.tile([128, NC_N, HO, B], F32, tag="fire")
                    rb = out_b[:].rearrange("c p ho b -> p c ho b")
                    for q in range(4):
                        nc.sync.dma_start(
                            fire_new[:, 2 * q:2 * q + 2, :, :],
                            rb[:, 2 * q:2 * q + 2, :, :])
                else:
                    fire_new = fire_prev

                fire_prev = fire_new
                out_pending = t

            if out_pending is not None:
                emit_outproj(out_pending, fire_prev)

            if mode == 'rdma':
                wq1 = nc.gpsimd.wait_ge(rsem, 0).then_inc(wkeep, 1)
                wq2 = nc.gpsimd.wait_ge(lsem, 0).then_inc(wkeep, 1)
                wq1.ins.add_nosync_dependencies_from(_names(trig))
                wq2.ins.add_nosync_dependencies_from(_names(wq1))
                wait_fixups.append((wq1.ins.name, rsem, 16 * t_steps))
                wait_fixups.append((wq2.ins.name, lsem, 16 * t_steps))

    nc.compile()
    if mode == 'rdma':
        fix = {n: (sem, val) for n, sem, val in wait_fixups}
        done = 0
        for f in nc.m.functions:
            for bb in f.blocks:
                for i in bb.instructions:
                    if i.name in fix:
                        sem, val = fix[i.name]
                        w = mybir.SyncWait(
                            sync_type='semaphore', id=sem.num,
                            ant_name=sem.name, wait_mode='sem-ge-imm',
                            wait_value=val)
                        si = i.sync_info
                        keep = [x for x in (si.on_wait if si else [])
                                if not (x.id == sem.num)]
                        i.sync_info = mybir.SyncInfo(
                            on_wait=keep + [w],
                            on_update=list(si.on_update) if si else [])
                        done += 1
        assert done == len(fix), f"wait fixups applied {done}/{len(fix)}"
    return nc


# ---------------- host-side data prep ----------------
def prep_inputs(inp, W_iv, b_iv, W_out, b_out, thresh, k_m, asc_amp, asc_r,
                asc_k, t_steps=T):
    inp = np.asarray(inp, np.float32)
    W_iv = np.asarray(W_iv, np.float32)
    b_iv = np.asarray(b_iv, np.float32)
    W_out = np.asarray(W_out, np.float32)
    b_out = np.asarray(b_out, np.float32)
    thresh = np.asarray(thresh, np.float32).reshape(HID)
    k_m = np.asarray(k_m, np.float32).reshape(HID)
    asc_amp = np.asarray(asc_amp, np.float32).reshape(A, HID)
    asc_r = np.asarray(asc_r, np.float32).reshape(A, HID)
    asc_k = np.asarray(asc_k, np.float32).reshape(A, HID)

    W_ix = W_iv[:, :IN]          # (HID, IN)
    W_hh = W_iv[:, IN:]          # (HID, HID)
    dec = np.exp(np.float32(-DT) * asc_k).astype(np.float32)   # (A, HID)
    km1 = (1.0 - np.float32(DT) * k_m).astype(np.float32)      # (HID,)

    # xT: (IN, T, B) bf16  — same for all cores
    xT = np.ascontiguousarray(inp[:, :t_steps, :].transpose(2, 1, 0))

    in_maps = []
    for c in range(NC_N):
        hs = slice(c * HL, (c + 1) * HL)   # own h rows (global order!)
        # wix[p_k, ki, ho, p_m] = W_ix[c*HL + ho*128 + p_m, ki*128 + p_k]
        wix = W_ix[hs].reshape(HO, 128, NKI, 128).transpose(3, 2, 0, 1)
        whh = W_hh[hs].reshape(HO, 128, NK, 128).transpose(3, 2, 0, 1)
        os_ = slice(c * OL, (c + 1) * OL)
        wo = W_out[os_].reshape(OL, NK, 128).transpose(2, 1, 0)
        shp = lambda x: x[hs].reshape(HO, 128).T.copy()           # [128, HO]
        shpb = lambda x: np.repeat(
            x[hs].reshape(HO, 128).T[:, :, None], B, axis=2)      # [128,HO,B]
        m = {
            "xT": xT,
            "wix": np.ascontiguousarray(wix, np.float32),
            "whh": np.ascontiguousarray(whh, np.float32),
            "wout": np.ascontiguousarray(wo, np.float32),
            "rcol": np.stack([shp(asc_r[a]) for a in range(A)], axis=1).copy(),
            "deccol": np.stack([shp(dec[a]) for a in range(A)], axis=1).copy(),
            "ampt": np.stack([shpb(asc_amp[a]) for a in range(A)], axis=1).copy(),
            "km1t": shpb(km1),
            "negth": shp(-thresh),
            "bivt": shpb(b_iv),
            "boutcol": b_out[os_].reshape(OL, 1).copy(),
        }
        in_maps.append(m)
    return in_maps


def assemble_output(results, t_steps=T):
    # per-core out: [t_steps, OL, B] -> full (B, T, OUT)
    full = np.empty((B, t_steps, OUT), np.float32)
    for c, r in enumerate(results):
        o = r["out"]                      # (t, OL, B)
        full[:, :, c * OL:(c + 1) * OL] = o.transpose(2, 0, 1)
    return full


_CACHE = {}


def _get_nc(t_steps=T):
    if t_steps not in _CACHE:
        _CACHE[t_steps] = build(t_steps)
    return _CACHE[t_steps]


def kernel(inp, W_iv, b_iv, W_out, b_out, thresh, k_m, asc_amp, asc_r, asc_k,
           t_steps=T):
    nc = _get_nc(t_steps)
    in_maps = prep_inputs(inp, W_iv, b_iv, W_out, b_out, thresh, k_m,
                          asc_amp, asc_r, asc_k, t_steps)
    res = bass_utils.run_bass_kernel_spmd(
        nc, in_maps, core_ids=list(range(NC_N)), trace=False)
    return assemble_output(res.results, t_steps)
